# revision 8
# baseline (speedup 1.0000x reference)
"""Causal self-attention + residual + LayerNorm — Trainium2, v3.

Problem: B=4, S=2048, D=1024, H=16 heads (hd=64), fp32 in/out.
Sharding: zig-zag, zero communication — core c -> batch c % 4, query-group
c // 4; g=0 owns query blocks 0 and 3, g=1 owns blocks 1 and 2.

v3 design (from hw trace evidence: dense matmuls hit 220 ns cadence =
full clock; attention + phase serialization were the loss):
  * All matmuls bf16 (same PE rate as f32r, half the SBUF/DMA);
    fp8 split schemes dropped — DoubleRow packs 2 k-tiles per
    instruction but per-instruction time is constant, so 3-term
    split-fp8 was 1.5x MORE instructions than plain bf16.
  * One resident x^T bf16 tile [P, DK, S] feeds the K, V and Q
    projections AND the residual add (queries are contiguous column
    chunks under the zig-zag layout).
  * Causal mask via DVE multiply on the bf16 exp tile (4x DVE mode:
    all-SBUF 2-byte operands) — zero PE mask work.
  * exp: one Act instruction per k-tile pair [128, 2, 512] PSUM->SBUF.
  * Softmax denominator from an appended ones column in V.
  * V bias folded into out-proj bias host-side (bo_eff = bo + Wo@bv).
  * Emission interleaving: K/V/Q projection groups are emitted between
    attention heads of q-tile 0, out-proj(qt0)+LN between the first
    heads of attention(qt1), keeping the PE dense (p-state ramped) and
    hiding everything behind the Act-bound attention windows.
"""
import sys

if "/opt/trn_rl_repo" not in sys.path:
    sys.path.insert(0, "/opt/trn_rl_repo")

import numpy as np
import ml_dtypes

B, S, D, H, HD = 4, 2048, 1024, 16, 64
P = 128
QT = 512
NQ = 1024
NKT = S // P                  # 16
DK = D // P                   # 8
NPLAIN = {0: (0, 12), 1: (4, 8)}   # group -> per-q-tile plain k-tiles
QCHUNK = {0: (0, 3), 1: (1, 2)}    # group -> global 512-query chunks owned
MOFF = [0, 128, 384, 768]          # column offsets of per-band-idx masks
ESCALE = 0.125                # 1/sqrt(hd)

BB = np.dtype(ml_dtypes.bfloat16)

_cache = {}


def _build():
    import concourse.mybir as mybir
    import concourse.tile as tile
    from concourse import bacc
    from concourse.bass import ts
    from concourse.alu_op_type import AluOpType

    f32 = mybir.dt.float32
    f32r = mybir.dt.float32r
    bf16 = mybir.dt.bfloat16
    AF = mybir.ActivationFunctionType

    nc = bacc.Bacc("TRN2", target_bir_lowering=False, debug=False, num_devices=8)

    xbd = nc.dram_tensor("xb", [D, S], bf16, kind="ExternalInput").ap()
    wkbd = nc.dram_tensor("wkb", [D, D], bf16, kind="ExternalInput").ap()
    wqbd = nc.dram_tensor("wqb", [D, D], bf16, kind="ExternalInput").ap()
    wvbd = nc.dram_tensor("wvb", [D, D], bf16, kind="ExternalInput").ap()
    wobd = nc.dram_tensor("wob", [D, D], bf16, kind="ExternalInput").ap()
    maskbd = nc.dram_tensor("maskb", [P, 1280], bf16, kind="ExternalInput").ap()
    bqd = nc.dram_tensor("bqd", [D], f32, kind="ExternalInput").ap()
    bkd = nc.dram_tensor("bkd", [D], f32, kind="ExternalInput").ap()
    bod = nc.dram_tensor("bod", [D], f32, kind="ExternalInput").ap()
    gamd = nc.dram_tensor("gamd", [D], f32, kind="ExternalInput").ap()
    betd = nc.dram_tensor("betd", [D], f32, kind="ExternalInput").ap()
    yt = nc.dram_tensor("yt", [D, NQ], f32, kind="ExternalOutput").ap()

    xb_r = xbd.rearrange("(dk p) t -> p dk t", p=P)

    with tile.TileContext(nc) as tc:
        with (
            tc.tile_pool(name="persist", bufs=1) as pers,
            tc.tile_pool(name="pp_ps", bufs=2, space="PSUM") as pp_ps,
            tc.tile_pool(name="s_ps", bufs=2, space="PSUM") as s_ps,
            tc.tile_pool(name="c_ps", bufs=2, space="PSUM") as c_ps,
        ):
            kt = pers.tile([P, DK, S], bf16)            # K^T       32 KB/part
            v8 = pers.tile([P, NKT, H, HD + 1], bf16)   # V + ones  32.5 KB
            xt = pers.tile([P, DK, S], bf16)            # x^T       32 KB
            mkb = pers.tile([P, 1280], bf16)            # causal masks 2.5 KB
            bia = pers.tile([P, DK, 5], f32)            # bq bk bo gam bet
            ones128 = pers.tile([P, 1], bf16)
            eps_t = pers.tile([1, 1], f32)
            nc.vector.memset(eps_t[:], 1e-5)
            nc.vector.memset(ones128[:], 1.0)
            nc.vector.memset(v8[:, :, :, HD], 1.0)

            for t in range(4):
                for hf in range(2):
                    sl = slice(4 * hf, 4 * hf + 4)
                    if t < 2:
                        eng = nc.sync if hf == 0 else nc.scalar
                    else:
                        eng = nc.gpsimd
                    eng.dma_start(xt[:, sl, ts(t, QT)], xb_r[:, sl, ts(t, QT)])
            nc.sync.dma_start(mkb[:], maskbd[:])
            for j, src in enumerate((bqd, bkd, bod, gamd, betd)):
                nc.sync.dma_start(bia[:, :, j], src.rearrange("(f p) -> p f", p=P))

            def bq_(f): return bia[:, f, 0:1]
            def bk_(f): return bia[:, f, 1:2]
            def bo_(f): return bia[:, f, 2:3]
            def gam_(f): return bia[:, f, 3:4]
            def bet_(f): return bia[:, f, 4:5]

            with (
                tc.tile_pool(name="qtp", bufs=1) as qtp,
                tc.tile_pool(name="ctxp", bufs=2) as ctxp,
                tc.tile_pool(name="sep", bufs=2) as sep,
                tc.tile_pool(name="scr", bufs=1) as scr,
                tc.tile_pool(name="wqp", bufs=2) as wqp,
                tc.tile_pool(name="ep", bufs=1) as ep,
            ):
                def kproj_group(t, f, wk):
                    ps = pp_ps.tile([P, QT], f32, tag="pp")
                    for dk in range(DK):
                        nc.tensor.matmul(
                            ps[:], wk[:, dk, f, :], xt[:, dk, ts(t, QT)],
                            start=(dk == 0), stop=(dk == DK - 1))
                    nc.vector.tensor_scalar_add(kt[:, f, ts(t, QT)], ps[:], bk_(f))

                def vproj_group(t, fg, wv):
                    ps = pp_ps.tile([P, QT], f32, tag="pp")
                    for dk in range(DK):
                        nc.tensor.matmul(
                            ps[:], xt[:, dk, ts(t, P)], wv[:, dk, fg, :],
                            start=(dk == 0), stop=(dk == DK - 1))
                    nc.vector.tensor_copy(v8[:, t, 8 * fg:8 * fg + 8, 0:HD], ps[:])

                def qproj_group(qc, f, qtile):
                    wq = wqp.tile([P, DK, P], bf16, tag="wq")
                    nc.sync.dma_start(
                        wq[:], wqbd[:, ts(f, P)].rearrange("(dk p) c -> p dk c", p=P))
                    ps = pp_ps.tile([P, QT], f32, tag="pp")
                    for dk in range(DK):
                        nc.tensor.matmul(
                            ps[:], wq[:, dk, :], xt[:, dk, ts(qc, QT)],
                            start=(dk == 0), stop=(dk == DK - 1))
                    nc.vector.tensor_scalar_add(qtile[:, f, :], ps[:], bq_(f))

                def attn_head(h, n_plain, qtile, ctx):
                    nk = n_plain + 4
                    nu = nk // 2
                    hp, base = h // 2, HD * (h % 2)
                    cp = c_ps.tile([HD + 1, QT], f32, tag="cp")

                    def scores(u):
                        sp = s_ps.tile([P, 2, QT], f32, tag="sp")
                        se = sep.tile([P, 2, QT], bf16, tag="se")
                        for j in (0, 1):
                            i = 2 * u + j
                            nc.tensor.matmul(
                                sp[:, j, :],
                                kt[base:base + HD, hp, ts(i, P)],
                                qtile[base:base + HD, hp, :],
                                start=True, stop=True)
                        nc.scalar.activation(se[:], sp[:], AF.Exp, scale=ESCALE)
                        if 2 * u >= n_plain:
                            bp = u - n_plain // 2
                            for j in (0, 1):
                                idx = 2 * bp + j
                                off, w = MOFF[idx], P * (idx + 1)
                                nc.vector.tensor_mul(
                                    se[:, j, 0:w], se[:, j, 0:w],
                                    mkb[:, off:off + w])
                        return se

                    # software pipeline: scores(u+1) issue ahead of ctx(u)
                    # so the PE never blocks on the exp feedback
                    se_cur = scores(0)
                    for u in range(nu):
                        se_nxt = scores(u + 1) if u + 1 < nu else None
                        for j in (0, 1):
                            i = 2 * u + j
                            nc.tensor.matmul(
                                cp[:], v8[:, i, h, :], se_cur[:, j, :],
                                start=(i == 0), stop=(i == nk - 1))
                        se_cur = se_nxt
                    den = scr.tile([1, QT], f32, tag="den")
                    nc.vector.tensor_copy(den[:], cp[HD:HD + 1, :])
                    rec = scr.tile([1, QT], f32, tag="rec")
                    rscr = scr.tile([1, QT], f32, tag="rscr")
                    nc.vector.reciprocal_approx_accurate(rec[:], den[:], rscr[:])
                    bc = scr.tile([HD, QT], f32, tag="bc")
                    nc.gpsimd.partition_broadcast(bc[:], rec[:])
                    po, ft = HD * (h % 2), h // 2
                    nc.vector.tensor_mul(ctx[po:po + HD, ft, :], cp[0:HD, :], bc[:])

                def out_group(qc, o, ctx, y, wob):
                    ps = pp_ps.tile([P, QT], f32, tag="pp")
                    for dk in range(DK):
                        nc.tensor.matmul(
                            ps[:], wob[:, dk, o, :], ctx[:, dk, :],
                            start=(dk == 0), stop=(dk == DK - 1))
                    nc.vector.scalar_tensor_tensor(
                        y[:, o, :], ps[:], bo_(o), xt[:, o, ts(qc, QT)],
                        AluOpType.add, AluOpType.add)

                def ln_stats(y):
                    mu_t = pp_ps.tile([P, QT], f32, tag="pp")
                    mu_ps = mu_t[0:1, :]
                    for o in range(DK):
                        nc.tensor.matmul(
                            mu_ps, ones128[:], y[:, o, :],
                            start=(o == 0), stop=(o == DK - 1))
                    ms_t = pp_ps.tile([P, QT], f32, tag="pp")
                    ms_ps = ms_t[0:1, :]
                    for o in range(DK):
                        ysq = ep.tile([P, QT], bf16, tag="ysq", bufs=2)
                        nc.vector.tensor_mul(ysq[:], y[:, o, :], y[:, o, :])
                        nc.tensor.matmul(
                            ms_ps, ones128[:], ysq[:],
                            start=(o == 0), stop=(o == DK - 1))
                    return mu_ps, ms_ps

                def ln_finish(qt, y, mu_ps, ms_ps):
                    mu = ep.tile([1, QT], f32, tag="mu_sb")
                    nc.vector.tensor_scalar_mul(mu[:], mu_ps, 1.0 / D)
                    ms = ep.tile([1, QT], f32, tag="ms_sb")
                    nc.vector.tensor_scalar_mul(ms[:], ms_ps, 1.0 / D)
                    tmp = ep.tile([1, QT], f32, tag="stat_tmp", bufs=2)
                    nc.vector.tensor_mul(tmp[:], mu[:], mu[:])
                    nc.vector.tensor_sub(ms[:], ms[:], tmp[:])  # var
                    sd = ep.tile([1, QT], f32, tag="stat_tmp", bufs=2)
                    nc.scalar.activation(sd[:], ms[:], AF.Sqrt, bias=eps_t[:])
                    rstd = ep.tile([1, QT], f32, tag="rstd")
                    rsc = ep.tile([1, QT], f32, tag="stat_tmp", bufs=2)
                    nc.vector.reciprocal_approx_accurate(rstd[:], sd[:], rsc[:])
                    mu_bc = ep.tile([P, QT], f32, tag="mu_bc")
                    nc.gpsimd.partition_broadcast(mu_bc[:], mu[:])
                    rs_bc = ep.tile([P, QT], f32, tag="rs_bc")
                    nc.gpsimd.partition_broadcast(rs_bc[:], rstd[:])
                    for o in range(DK):
                        t1 = ep.tile([P, QT], f32, tag="t1", bufs=2)
                        nc.vector.tensor_sub(t1[:], y[:, o, :], mu_bc[:])
                        nc.vector.tensor_mul(t1[:], t1[:], rs_bc[:])
                        yo = ep.tile([P, QT], f32, tag="yo", bufs=2)
                        nc.vector.tensor_scalar(
                            yo[:], t1[:], gam_(o), bet_(o),
                            AluOpType.mult, AluOpType.add)
                        nc.sync.dma_start(yt[ts(o, P), ts(qt, QT)], yo[:])

                def group(g):
                    qc0, qc1 = QCHUNK[g]
                    nkc0 = 1 if g == 0 else 2   # kt chunks needed by attn0
                    nv0 = NPLAIN[g][0] + 4      # v tiles needed by attn0

                    qt0 = qtp.tile([P, DK, QT], bf16, tag="qtile")
                    ctx0 = ctxp.tile([P, DK, QT], bf16, tag="ctx")

                    with tc.tile_pool(name="wkv", bufs=1) as wkvp:
                        wk = wkvp.tile([P, DK, DK, P], bf16)
                        wv = wkvp.tile([P, DK, 2, 512], bf16)
                        wk_r = wkbd.rearrange(
                            "(dk p) (f c) -> p dk f c", p=P, c=P)
                        for fz in range(4):
                            eng = nc.scalar if fz % 2 else nc.sync
                            eng.dma_start(
                                wk[:, :, 2 * fz:2 * fz + 2, :],
                                wk_r[:, :, 2 * fz:2 * fz + 2, :])
                        wv_r = wvbd.rearrange(
                            "(dk p) (g c) -> p dk g c", p=P, c=512)
                        for fg in range(2):
                            nc.scalar.dma_start(wv[:, :, fg, :], wv_r[:, :, fg, :])

                        for t in range(nkc0):
                            for f in range(DK):
                                kproj_group(t, f, wk)
                        for f in range(DK):
                            qproj_group(qc0, f, qt0)
                        for t in range(nv0):
                            for fg in range(2):
                                vproj_group(t, fg, wv)

                        # filler: remaining projection groups, emitted between
                        # attention heads to keep the PE dense
                        nkc = 4 if g == 0 else 3
                        nvt = NKT if g == 0 else 12
                        filler = []
                        for t in range(nkc0, nkc):
                            for f in range(DK):
                                filler.append((kproj_group, (t, f, wk)))
                        for t in range(nv0, nvt):
                            for fg in range(2):
                                filler.append((vproj_group, (t, fg, wv)))
                        fi = iter(filler)

                        def take(n):
                            for _ in range(n):
                                fn, args = next(fi, (None, None))
                                if fn is None:
                                    return
                                fn(*args)

                        per_head = (len(filler) + H - 1) // H
                        for h in range(H):
                            attn_head(h, NPLAIN[g][0], qt0, ctx0)
                            take(per_head)
                        take(len(filler))
                    qt1 = qtp.tile([P, DK, QT], bf16, tag="qtile")
                    for f in range(DK):
                        qproj_group(qc1, f, qt1)

                    # attn(qt1) with out-proj(qt0) + LN(qt0) as filler
                    with tc.tile_pool(name="wo2", bufs=1) as wo2p:
                        wob = wo2p.tile([P, DK, DK, P], bf16)
                        wo_r = wobd.rearrange(
                            "(dk p) (o c) -> p dk o c", p=P, c=P)
                        for oz in range(4):
                            nc.sync.dma_start(
                                wob[:, :, 2 * oz:2 * oz + 2, :],
                                wo_r[:, :, 2 * oz:2 * oz + 2, :])
                        ctx1 = ctxp.tile([P, DK, QT], bf16, tag="ctx")
                        y0 = ep.tile([P, DK, QT], bf16, tag="y")
                        for h in range(H):
                            attn_head(h, NPLAIN[g][1], qt1, ctx1)
                            if h < DK:
                                out_group(qc0, h, ctx0, y0, wob)

                        # LN(qt0) in the tail: its Sqrt would thrash the Act
                        # table (Exp<->Sqrt reloads) mid-pipeline otherwise
                        mu0, ms0 = ln_stats(y0)
                        ln_finish(0, y0, mu0, ms0)
                        y1 = ep.tile([P, DK, QT], bf16, tag="y")
                        for o in range(DK):
                            out_group(qc1, o, ctx1, y1, wob)
                        mu1, ms1 = ln_stats(y1)
                        ln_finish(1, y1, mu1, ms1)

                pid = nc.partition_id()
                with tc.If(pid < 4) as cmp:
                    group(0)
                with cmp.Else():
                    group(1)
    nc.compile()
    return nc


def _get_nc():
    if "nc" not in _cache:
        _cache["nc"] = _build()
    return _cache["nc"]


def _prep(x, in_proj_w, in_proj_b, out_w, out_b, gamma, beta):
    x = np.asarray(x, np.float32)
    wt = np.ascontiguousarray(np.asarray(in_proj_w, np.float32).T)
    wot = np.ascontiguousarray(np.asarray(out_w, np.float32).T)
    bqkv = np.asarray(in_proj_b, np.float32)
    bo = np.asarray(out_b, np.float32)
    gam = np.asarray(gamma, np.float32)
    bet = np.asarray(beta, np.float32)

    wqb = np.ascontiguousarray(wt[:, 0:D].astype(BB))
    wkb = np.ascontiguousarray(wt[:, D:2 * D].astype(BB))
    wvb = np.ascontiguousarray(wt[:, 2 * D:3 * D].astype(BB))
    wob = wot.astype(BB)
    bo_eff = bo + wot.T @ bqkv[2 * D:3 * D]

    # per-band-idx causal masks, packed [P, 1280]:
    # idx block at MOFF[idx], width 128*(idx+1); valid iff c >= 128*idx + p
    p = np.arange(P)[:, None]
    mk = np.zeros((P, 1280), np.float32)
    for idx in range(4):
        w = P * (idx + 1)
        c = np.arange(w)[None, :]
        mk[:, MOFF[idx]:MOFF[idx] + w] = (c >= P * idx + p)
    maskb = np.ascontiguousarray(mk.astype(BB))

    qcols = {
        0: np.r_[0:QT, 3 * QT:4 * QT],
        1: np.r_[QT:3 * QT],
    }
    in_maps = []
    for cidx in range(8):
        b = cidx % 4
        xbt = np.ascontiguousarray(x[b].T.astype(BB))
        in_maps.append({
            "xb": xbt,
            "wqb": wqb, "wkb": wkb, "wvb": wvb, "wob": wob,
            "maskb": maskb,
            "bqd": bqkv[0:D], "bkd": bqkv[D:2 * D], "bod": bo_eff,
            "gamd": gam, "betd": bet,
        })
    return in_maps, qcols


def _run(in_maps, trace=False, **kw):
    from concourse.bass_utils import run_bass_kernel_spmd

    return run_bass_kernel_spmd(_get_nc(), in_maps, list(range(8)), trace=trace, **kw)


def kernel(x, in_proj_w, in_proj_b, out_w, out_b, gamma, beta):
    in_maps, qcols = _prep(x, in_proj_w, in_proj_b, out_w, out_b, gamma, beta)
    res = _run(in_maps)
    out = np.empty((B, S, D), np.float32)
    for c in range(8):
        out[c % 4, qcols[c // 4]] = res.results[c]["yt"].T
    return out


# revision 11
# speedup vs baseline: 1.0275x; 1.0275x over previous
"""Causal self-attention + residual + LayerNorm — Trainium2, v3.

Problem: B=4, S=2048, D=1024, H=16 heads (hd=64), fp32 in/out.
Sharding: zig-zag, zero communication — core c -> batch c % 4, query-group
c // 4; g=0 owns query blocks 0 and 3, g=1 owns blocks 1 and 2.

Design (driven by hw traces: dense matmuls hit a ~220 ns cadence = full
clock; sparse PE queues downclock ~2x; per-instruction time is constant
regardless of dtype, so minimize instruction count and keep PE dense):
  * All matmuls bf16 (same PE rate as f32r, half the SBUF/DMA traffic;
    fp8-DoubleRow/split schemes measured slower or too lossy).
  * One resident x^T bf16 tile [P, DK, S] feeds the K, V and Q
    projections AND the residual add (queries are contiguous column
    chunks under the zig-zag layout).
  * Causal mask via DVE multiply on the bf16 exp tile, restricted to
    the minimal masked column ranges — zero PE mask work.
  * exp: one Act instruction per k-tile pair [128, 2, 512] PSUM->SBUF.
  * Attention inner loop is software-pipelined: scores(u+1) is emitted
    before ctx(u) so the in-order PE never blocks on the exp feedback.
  * Softmax denominator from an appended ones column in V.
  * V bias folded into out-proj bias host-side (bo_eff = bo + Wo@bv).
  * Emission interleaving: K/V/Q projection groups are emitted between
    attention heads of q-tile 0, out-proj(qt0) between the first heads
    of attention(qt1); both LayerNorms run in the tail so their Sqrt
    never thrashes the Act engine's Exp table mid-pipeline.
  * Big DMAs split into consumption-ordered pieces across the SP and
    Act engine rings; out-proj weights resident during phase 2.
"""
import sys

if "/opt/trn_rl_repo" not in sys.path:
    sys.path.insert(0, "/opt/trn_rl_repo")

import numpy as np
import ml_dtypes

B, S, D, H, HD = 4, 2048, 1024, 16, 64
P = 128
QT = 512
NQ = 1024
NKT = S // P                  # 16
DK = D // P                   # 8
NPLAIN = {0: (0, 12), 1: (4, 8)}   # group -> per-q-tile plain k-tiles
QCHUNK = {0: (0, 3), 1: (1, 2)}    # group -> global 512-query chunks owned
MOFF = [0, 128, 384, 768]          # column offsets of per-band-idx masks
ESCALE = 0.125                # 1/sqrt(hd)

BB = np.dtype(ml_dtypes.bfloat16)

_cache = {}


def _build():
    import concourse.mybir as mybir
    import concourse.tile as tile
    from concourse import bacc
    from concourse.bass import ts
    from concourse.alu_op_type import AluOpType

    f32 = mybir.dt.float32
    f32r = mybir.dt.float32r
    bf16 = mybir.dt.bfloat16
    AF = mybir.ActivationFunctionType

    nc = bacc.Bacc("TRN2", target_bir_lowering=False, debug=False, num_devices=8)

    xbd = nc.dram_tensor("xb", [D, S], bf16, kind="ExternalInput").ap()
    wkbd = nc.dram_tensor("wkb", [D, D], bf16, kind="ExternalInput").ap()
    wqbd = nc.dram_tensor("wqb", [D, D], bf16, kind="ExternalInput").ap()
    wvbd = nc.dram_tensor("wvb", [D, D], bf16, kind="ExternalInput").ap()
    wobd = nc.dram_tensor("wob", [D, D], bf16, kind="ExternalInput").ap()
    maskbd = nc.dram_tensor("maskb", [P, 1280], bf16, kind="ExternalInput").ap()
    bqd = nc.dram_tensor("bqd", [D], f32, kind="ExternalInput").ap()
    bkd = nc.dram_tensor("bkd", [D], f32, kind="ExternalInput").ap()
    bod = nc.dram_tensor("bod", [D], f32, kind="ExternalInput").ap()
    gamd = nc.dram_tensor("gamd", [D], f32, kind="ExternalInput").ap()
    betd = nc.dram_tensor("betd", [D], f32, kind="ExternalInput").ap()
    yt = nc.dram_tensor("yt", [D, NQ], f32, kind="ExternalOutput").ap()

    xb_r = xbd.rearrange("(dk p) t -> p dk t", p=P)

    with tile.TileContext(nc) as tc:
        with (
            tc.tile_pool(name="persist", bufs=1) as pers,
            tc.tile_pool(name="pp_ps", bufs=2, space="PSUM") as pp_ps,
            tc.tile_pool(name="s_ps", bufs=2, space="PSUM") as s_ps,
            tc.tile_pool(name="c_ps", bufs=2, space="PSUM") as c_ps,
        ):
            kt = pers.tile([P, DK, S], bf16)            # K^T       32 KB/part
            v8 = pers.tile([P, NKT, H, HD + 1], bf16)   # V + ones  32.5 KB
            xt = pers.tile([P, DK, S], bf16)            # x^T       32 KB
            mkb = pers.tile([P, 1280], bf16)            # causal masks 2.5 KB
            bia = pers.tile([P, DK, 5], f32)            # bq bk bo gam bet
            ones128 = pers.tile([P, 1], bf16)
            eps_t = pers.tile([1, 1], f32)
            nc.vector.memset(eps_t[:], 1e-5)
            nc.vector.memset(ones128[:], 1.0)
            nc.vector.memset(v8[:, :, :, HD], 1.0)

            for t in range(2):
                for hf in range(2):
                    sl = slice(4 * hf, 4 * hf + 4)
                    eng = nc.sync if hf == 0 else nc.scalar
                    eng.dma_start(xt[:, sl, ts(t, QT)], xb_r[:, sl, ts(t, QT)])
            nc.sync.dma_start(mkb[:], maskbd[:])
            for j, src in enumerate((bqd, bkd, bod, gamd, betd)):
                nc.sync.dma_start(bia[:, :, j], src.rearrange("(f p) -> p f", p=P))

            def bq_(f): return bia[:, f, 0:1]
            def bk_(f): return bia[:, f, 1:2]
            def bo_(f): return bia[:, f, 2:3]
            def gam_(f): return bia[:, f, 3:4]
            def bet_(f): return bia[:, f, 4:5]

            with (
                tc.tile_pool(name="qtp", bufs=1) as qtp,
                tc.tile_pool(name="ctxp", bufs=2) as ctxp,
                tc.tile_pool(name="sep", bufs=2) as sep,
                tc.tile_pool(name="scr", bufs=1) as scr,
                tc.tile_pool(name="wqp", bufs=2) as wqp,
                tc.tile_pool(name="ep", bufs=1) as ep,
            ):
                def kproj_group(t, f, wk):
                    ps = pp_ps.tile([P, QT], f32, tag="pp")
                    for dk in range(DK):
                        nc.tensor.matmul(
                            ps[:], wk[:, dk, f, :], xt[:, dk, ts(t, QT)],
                            start=(dk == 0), stop=(dk == DK - 1))
                    nc.vector.tensor_scalar_add(kt[:, f, ts(t, QT)], ps[:], bk_(f))

                def vproj_group(t, fg, wv):
                    ps = pp_ps.tile([P, QT], f32, tag="pp")
                    for dk in range(DK):
                        nc.tensor.matmul(
                            ps[:], xt[:, dk, ts(t, P)], wv[:, dk, fg, :],
                            start=(dk == 0), stop=(dk == DK - 1))
                    nc.vector.tensor_copy(v8[:, t, 8 * fg:8 * fg + 8, 0:HD], ps[:])

                def qproj_group(qc, f, qtile):
                    wq = wqp.tile([P, DK, P], bf16, tag="wq")
                    nc.sync.dma_start(
                        wq[:], wqbd[:, ts(f, P)].rearrange("(dk p) c -> p dk c", p=P))
                    ps = pp_ps.tile([P, QT], f32, tag="pp")
                    for dk in range(DK):
                        nc.tensor.matmul(
                            ps[:], wq[:, dk, :], xt[:, dk, ts(qc, QT)],
                            start=(dk == 0), stop=(dk == DK - 1))
                    nc.vector.tensor_scalar_add(qtile[:, f, :], ps[:], bq_(f))

                def attn_head(h, n_plain, qtile, ctx):
                    nk = n_plain + 4
                    nu = nk // 2
                    hp, base = h // 2, HD * (h % 2)
                    cp = c_ps.tile([HD + 1, QT], f32, tag="cp")

                    def scores(u):
                        sp = s_ps.tile([P, 2, QT], f32, tag="sp")
                        se = sep.tile([P, 2, QT], bf16, tag="se")
                        for j in (0, 1):
                            i = 2 * u + j
                            nc.tensor.matmul(
                                sp[:, j, :],
                                kt[base:base + HD, hp, ts(i, P)],
                                qtile[base:base + HD, hp, :],
                                start=True, stop=True)
                        nc.scalar.activation(se[:], sp[:], AF.Exp, scale=ESCALE)
                        if 2 * u >= n_plain:
                            bp = u - n_plain // 2
                            for j in (0, 1):
                                idx = 2 * bp + j
                                off, w = MOFF[idx], P * (idx + 1)
                                nc.vector.tensor_mul(
                                    se[:, j, 0:w], se[:, j, 0:w],
                                    mkb[:, off:off + w])
                        return se

                    # software pipeline: scores(u+1) issue ahead of ctx(u)
                    # so the PE never blocks on the exp feedback
                    se_cur = scores(0)
                    for u in range(nu):
                        se_nxt = scores(u + 1) if u + 1 < nu else None
                        for j in (0, 1):
                            i = 2 * u + j
                            nc.tensor.matmul(
                                cp[:], v8[:, i, h, :], se_cur[:, j, :],
                                start=(i == 0), stop=(i == nk - 1))
                        se_cur = se_nxt
                    den = scr.tile([1, QT], f32, tag="den")
                    nc.vector.tensor_copy(den[:], cp[HD:HD + 1, :])
                    rec = scr.tile([1, QT], f32, tag="rec")
                    rscr = scr.tile([1, QT], f32, tag="rscr")
                    nc.vector.reciprocal_approx_accurate(rec[:], den[:], rscr[:])
                    bc = scr.tile([HD, QT], f32, tag="bc")
                    nc.gpsimd.partition_broadcast(bc[:], rec[:])
                    po, ft = HD * (h % 2), h // 2
                    nc.vector.tensor_mul(ctx[po:po + HD, ft, :], cp[0:HD, :], bc[:])

                def out_group(qc, o, ctx, y, wob):
                    ps = pp_ps.tile([P, QT], f32, tag="pp")
                    for dk in range(DK):
                        nc.tensor.matmul(
                            ps[:], wob[:, dk, o, :], ctx[:, dk, :],
                            start=(dk == 0), stop=(dk == DK - 1))
                    nc.vector.scalar_tensor_tensor(
                        y[:, o, :], ps[:], bo_(o), xt[:, o, ts(qc, QT)],
                        AluOpType.add, AluOpType.add)

                def ln_stats(y):
                    mu_t = pp_ps.tile([P, QT], f32, tag="pp")
                    mu_ps = mu_t[0:1, :]
                    for o in range(DK):
                        nc.tensor.matmul(
                            mu_ps, ones128[:], y[:, o, :],
                            start=(o == 0), stop=(o == DK - 1))
                    ms_t = pp_ps.tile([P, QT], f32, tag="pp")
                    ms_ps = ms_t[0:1, :]
                    for o in range(DK):
                        ysq = ep.tile([P, QT], bf16, tag="ysq", bufs=2)
                        nc.vector.tensor_mul(ysq[:], y[:, o, :], y[:, o, :])
                        nc.tensor.matmul(
                            ms_ps, ones128[:], ysq[:],
                            start=(o == 0), stop=(o == DK - 1))
                    return mu_ps, ms_ps

                def ln_finish(qt, y, mu_ps, ms_ps):
                    mu = ep.tile([1, QT], f32, tag="mu_sb")
                    nc.vector.tensor_scalar_mul(mu[:], mu_ps, 1.0 / D)
                    ms = ep.tile([1, QT], f32, tag="ms_sb")
                    nc.vector.tensor_scalar_mul(ms[:], ms_ps, 1.0 / D)
                    tmp = ep.tile([1, QT], f32, tag="stat_tmp", bufs=2)
                    nc.vector.tensor_mul(tmp[:], mu[:], mu[:])
                    nc.vector.tensor_sub(ms[:], ms[:], tmp[:])  # var
                    sd = ep.tile([1, QT], f32, tag="stat_tmp", bufs=2)
                    nc.scalar.activation(sd[:], ms[:], AF.Sqrt, bias=eps_t[:])
                    rstd = ep.tile([1, QT], f32, tag="rstd")
                    rsc = ep.tile([1, QT], f32, tag="stat_tmp", bufs=2)
                    nc.vector.reciprocal_approx_accurate(rstd[:], sd[:], rsc[:])
                    mu_bc = ep.tile([P, QT], f32, tag="mu_bc")
                    nc.gpsimd.partition_broadcast(mu_bc[:], mu[:])
                    rs_bc = ep.tile([P, QT], f32, tag="rs_bc")
                    nc.gpsimd.partition_broadcast(rs_bc[:], rstd[:])
                    for o in range(DK):
                        t1 = ep.tile([P, QT], f32, tag="t1", bufs=2)
                        nc.vector.tensor_sub(t1[:], y[:, o, :], mu_bc[:])
                        nc.vector.tensor_mul(t1[:], t1[:], rs_bc[:])
                        yo = ep.tile([P, QT], f32, tag="yo", bufs=2)
                        nc.vector.tensor_scalar(
                            yo[:], t1[:], gam_(o), bet_(o),
                            AluOpType.mult, AluOpType.add)
                        nc.sync.dma_start(yt[ts(o, P), ts(qt, QT)], yo[:])

                def group(g):
                    qc0, qc1 = QCHUNK[g]
                    nkc0 = 1 if g == 0 else 2   # kt chunks needed by attn0
                    nv0 = NPLAIN[g][0] + 4      # v tiles needed by attn0

                    qt0 = qtp.tile([P, DK, QT], bf16, tag="qtile")
                    ctx0 = ctxp.tile([P, DK, QT], bf16, tag="ctx")

                    with tc.tile_pool(name="wkv", bufs=1) as wkvp:
                        wk = wkvp.tile([P, DK, DK, P], bf16)
                        wv = wkvp.tile([P, DK, 2, 512], bf16)
                        wk_r = wkbd.rearrange(
                            "(dk p) (f c) -> p dk f c", p=P, c=P)
                        for fz in range(4):
                            eng = nc.scalar if fz % 2 else nc.sync
                            eng.dma_start(
                                wk[:, :, 2 * fz:2 * fz + 2, :],
                                wk_r[:, :, 2 * fz:2 * fz + 2, :])
                        wv_r = wvbd.rearrange(
                            "(dk p) (g c) -> p dk g c", p=P, c=512)
                        for fg in range(2):
                            eng = nc.sync if fg else nc.scalar
                            eng.dma_start(wv[:, :, fg, :], wv_r[:, :, fg, :])

                        for t in range(nkc0):
                            for f in range(DK):
                                kproj_group(t, f, wk)
                        for f in range(DK):
                            qproj_group(qc0, f, qt0)
                        for t in range(nv0):
                            for fg in range(2):
                                vproj_group(t, fg, wv)

                        # late x chunks: consumed only by mid-attention
                        # fillers, so keep them off the startup rings
                        for t2 in (2, 3):
                            for hf in range(2):
                                sl = slice(4 * hf, 4 * hf + 4)
                                eng = nc.sync if hf == 0 else nc.scalar
                                eng.dma_start(
                                    xt[:, sl, ts(t2, QT)],
                                    xb_r[:, sl, ts(t2, QT)])

                        # filler: remaining projection groups, emitted between
                        # attention heads to keep the PE dense
                        nkc = 4 if g == 0 else 3
                        nvt = NKT if g == 0 else 12
                        filler = []
                        for t in range(nkc0, nkc):
                            for f in range(DK):
                                filler.append((kproj_group, (t, f, wk)))
                        for t in range(nv0, nvt):
                            for fg in range(2):
                                filler.append((vproj_group, (t, fg, wv)))
                        fi = iter(filler)

                        def take(n):
                            for _ in range(n):
                                fn, args = next(fi, (None, None))
                                if fn is None:
                                    return
                                fn(*args)

                        per_head = (len(filler) + H - 1) // H
                        for h in range(H):
                            attn_head(h, NPLAIN[g][0], qt0, ctx0)
                            take(per_head)
                        take(len(filler))
                    qt1 = qtp.tile([P, DK, QT], bf16, tag="qtile")
                    for f in range(DK):
                        qproj_group(qc1, f, qt1)

                    # attn(qt1) with out-proj(qt0) + LN(qt0) as filler
                    with tc.tile_pool(name="wo2", bufs=1) as wo2p:
                        wob = wo2p.tile([P, DK, DK, P], bf16)
                        wo_r = wobd.rearrange(
                            "(dk p) (o c) -> p dk o c", p=P, c=P)
                        for oz in range(4):
                            nc.sync.dma_start(
                                wob[:, :, 2 * oz:2 * oz + 2, :],
                                wo_r[:, :, 2 * oz:2 * oz + 2, :])
                        ctx1 = ctxp.tile([P, DK, QT], bf16, tag="ctx")
                        y0 = ep.tile([P, DK, QT], bf16, tag="y")
                        for h in range(H):
                            attn_head(h, NPLAIN[g][1], qt1, ctx1)
                            if h < DK:
                                out_group(qc0, h, ctx0, y0, wob)

                        # LN(qt0) in the tail: its Sqrt would thrash the Act
                        # table (Exp<->Sqrt reloads) mid-pipeline otherwise
                        mu0, ms0 = ln_stats(y0)
                        ln_finish(0, y0, mu0, ms0)
                        y1 = ep.tile([P, DK, QT], bf16, tag="y")
                        for o in range(DK):
                            out_group(qc1, o, ctx1, y1, wob)
                        mu1, ms1 = ln_stats(y1)
                        ln_finish(1, y1, mu1, ms1)

                pid = nc.partition_id()
                with tc.If(pid < 4) as cmp:
                    group(0)
                with cmp.Else():
                    group(1)
    nc.compile()
    return nc


def _get_nc():
    if "nc" not in _cache:
        _cache["nc"] = _build()
    return _cache["nc"]


def _prep(x, in_proj_w, in_proj_b, out_w, out_b, gamma, beta):
    x = np.asarray(x, np.float32)
    wt = np.ascontiguousarray(np.asarray(in_proj_w, np.float32).T)
    wot = np.ascontiguousarray(np.asarray(out_w, np.float32).T)
    bqkv = np.asarray(in_proj_b, np.float32)
    bo = np.asarray(out_b, np.float32)
    gam = np.asarray(gamma, np.float32)
    bet = np.asarray(beta, np.float32)

    wqb = np.ascontiguousarray(wt[:, 0:D].astype(BB))
    wkb = np.ascontiguousarray(wt[:, D:2 * D].astype(BB))
    wvb = np.ascontiguousarray(wt[:, 2 * D:3 * D].astype(BB))
    wob = wot.astype(BB)
    bo_eff = bo + wot.T @ bqkv[2 * D:3 * D]

    # per-band-idx causal masks, packed [P, 1280]:
    # idx block at MOFF[idx], width 128*(idx+1); valid iff c >= 128*idx + p
    p = np.arange(P)[:, None]
    mk = np.zeros((P, 1280), np.float32)
    for idx in range(4):
        w = P * (idx + 1)
        c = np.arange(w)[None, :]
        mk[:, MOFF[idx]:MOFF[idx] + w] = (c >= P * idx + p)
    maskb = np.ascontiguousarray(mk.astype(BB))

    qcols = {
        0: np.r_[0:QT, 3 * QT:4 * QT],
        1: np.r_[QT:3 * QT],
    }
    in_maps = []
    for cidx in range(8):
        b = cidx % 4
        xbt = np.ascontiguousarray(x[b].T.astype(BB))
        in_maps.append({
            "xb": xbt,
            "wqb": wqb, "wkb": wkb, "wvb": wvb, "wob": wob,
            "maskb": maskb,
            "bqd": bqkv[0:D], "bkd": bqkv[D:2 * D], "bod": bo_eff,
            "gamd": gam, "betd": bet,
        })
    return in_maps, qcols


def _run(in_maps, trace=False, **kw):
    from concourse.bass_utils import run_bass_kernel_spmd

    return run_bass_kernel_spmd(_get_nc(), in_maps, list(range(8)), trace=trace, **kw)


def kernel(x, in_proj_w, in_proj_b, out_w, out_b, gamma, beta):
    in_maps, qcols = _prep(x, in_proj_w, in_proj_b, out_w, out_b, gamma, beta)
    res = _run(in_maps)
    out = np.empty((B, S, D), np.float32)
    for c in range(8):
        out[c % 4, qcols[c // 4]] = res.results[c]["yt"].T
    return out
